# revision 12
# baseline (speedup 1.0000x reference)
"""Trainium2 Bass kernel for the DifferentiableAllocator (Sinkhorn) problem.

Math: the reference runs 200 log-domain Sinkhorn iterations on K = -(C-theta)/eps
with row marginal a and column marginal b.  With E = exp(K - rowmax(K)) and the
identities u = exp(f + rowmax(K)) = a / (E w),  w = exp(g) = b / (E^T u), the
whole loop collapses to scaled matrix-balancing with NO transcendentals:

    w0 = 1;  repeat 200x:  S = E w;  u = a/S;  T = E^T u;  w = b/T
    P = diag(u) E diag(w) / sum(...)

which is exactly what the reference computes (gauge-invariant; validated to
rel err ~5.6e-5 against the fp32 jax reference, 3.5e-6 against fp64).

Layout: rows L=8192 are folded onto 128 partitions x 64 rows; the 8 columns are
contiguous in the free dim ([128, (r=64, j=8)]).  Row reduce = strided free-dim
reduce (DVE); column reduce = free-dim reduce per column + ones-matmul partition
reduce (PE); w broadcast to partitions = rank-1 ones outer product (PE).

The whole problem runs replicated on all 8 cores (it is a single serial
200-iteration chain over just 256 KB of state - sharding would add an
allreduce to every iteration for no win); core 0's output is returned.
"""

import numpy as np
from contextlib import ExitStack

import concourse.bass as bass
import concourse.bacc as bacc
import concourse.tile as tile
from concourse import mybir
from concourse.bass_utils import run_bass_kernel_spmd

L, B = 8192, 8
P = 128
R = L // P  # 64 rows per partition
ITERS = 200
EPS_INV = 50.0  # 1/0.02
BITS = (2.0, 3.0, 4.0, 5.0, 6.0, 8.0, 10.0, 12.0)
F32 = mybir.dt.float32
ADD = mybir.AluOpType.add
AXX = mybir.AxisListType.X


def build(iters=ITERS):
    nc = bacc.Bacc("TRN2", target_bir_lowering=False, debug=False)
    theta_d = nc.dram_tensor("theta", [L, B], F32, kind="ExternalInput").ap()
    phi_d = nc.dram_tensor("phi", [B], F32, kind="ExternalInput").ap()
    sens_d = nc.dram_tensor("sens_raw", [L], F32, kind="ExternalInput").ap()
    nraw_d = nc.dram_tensor("n_raw", [L], F32, kind="ExternalInput").ap()
    out_d = nc.dram_tensor("out", [L, B], F32, kind="ExternalOutput").ap()

    with tile.TileContext(nc) as tc, ExitStack() as ctx:
        sb = ctx.enter_context(tc.tile_pool(name="sb", bufs=1))
        ps = ctx.enter_context(tc.tile_pool(name="ps", bufs=2, space="PSUM"))
        ps1 = ctx.enter_context(tc.tile_pool(name="ps1", bufs=1, space="PSUM"))

        # ---- persistent tiles ----
        TH = sb.tile([P, R * B], F32, tag="TH")    # theta, then scratch
        E = sb.tile([P, R * B], F32, tag="E")      # exp(K - rowmax K)
        PR = sb.tile([P, R * B], F32, tag="PR")    # E * w  (row-pass product)
        PR2 = sb.tile([P, R * B], F32, tag="PR2")  # E * u  (col-pass product)
        EB = sb.tile([P, R * B], F32, tag="EB")    # err broadcast, then scratch
        NR = sb.tile([P, R], F32, tag="NR")        # n_raw -> n
        SR = sb.tile([P, R], F32, tag="SR")        # sens_raw
        A = sb.tile([P, R], F32, tag="A")          # a = n / sum(n)
        S = sb.tile([P, R], F32, tag="S")
        RS = sb.tile([P, R], F32, tag="RS")        # 1/S
        SC = sb.tile([P, R], F32, tag="SC")        # recip scratch
        U = sb.tile([P, R], F32, tag="U")
        M = sb.tile([P, R], F32, tag="M")          # row min of D
        TP = sb.tile([P, B], F32, tag="TP")        # per-partition column partials
        W = sb.tile([1, B], F32, tag="W")
        RT = sb.tile([1, B], F32, tag="RT")
        BT = sb.tile([1, B], F32, tag="BT")        # b (normalized softmax(phi))
        PH = sb.tile([1, B], F32, tag="PH")
        SMALL = sb.tile([1, 4], F32, tag="SMALL")  # misc 1x1 values
        COLP = sb.tile([P, 2], F32, tag="COLP")    # per-partition partial sums
        SCL = sb.tile([P, 2], F32, tag="SCL")      # per-partition scalars (sbuf)
        ONESC = sb.tile([P, 1], F32, tag="ONESC")  # ones column  (matmul lhsT)
        ONESR = sb.tile([1, P], F32, tag="ONESR")  # ones row     (matmul lhsT)
        WB0 = sb.tile([P, B], F32, tag="WB0")      # w broadcast, iteration 0

        def r3(t):  # [P, R*B] -> [P, R, B]
            return t[:].rearrange("p (r j) -> p r j", j=B)

        def c3(t):  # [P, R*B] -> [P, B, R]  (column-major view for col reduce)
            return t[:].rearrange("p (r j) -> p j r", j=B)

        def bcast_j(t2):  # [P, R] -> [P, R, B] (step-0 over j)
            return t2[:].unsqueeze(2).broadcast_to((P, R, B))

        def bcast_r(t2):  # [P, B] -> [P, R, B] (step-0 over r)
            return t2[:].unsqueeze(1).broadcast_to((P, R, B))

        # ---- loads ----
        nc.sync.dma_start(TH[:], theta_d.rearrange("(p r) j -> p (r j)", p=P))
        nc.sync.dma_start(NR[:], nraw_d.rearrange("(p r) -> p r", p=P))
        nc.sync.dma_start(SR[:], sens_d.rearrange("(p r) -> p r", p=P))
        nc.sync.dma_start(PH[:], phi_d.unsqueeze(0))

        # ---- constants ----
        nc.gpsimd.memset(ONESC[:], 1.0)
        nc.gpsimd.memset(ONESR[:], 1.0)
        nc.gpsimd.memset(WB0[:], 1.0)
        for j, bits in enumerate(BITS):
            nc.gpsimd.memset(r3(EB)[:, :, j], float(2.0 ** (-2.0 * bits)))

        # ---- setup: n, sens, a, b, E ----
        # n = n_raw * 1e5 + 1e3   (in place)
        nc.vector.tensor_scalar(NR[:], NR[:], 1e5, 1e3,
                                op0=mybir.AluOpType.mult, op1=ADD)
        # sum over all partitions: free-reduce then ones-matmul
        nc.vector.tensor_reduce(COLP[:, 0:1], SR[:], axis=AXX, op=ADD)
        nc.vector.tensor_reduce(COLP[:, 1:2], NR[:], axis=AXX, op=ADD)
        PSS = ps1.tile([1, 2], F32, tag="small1")
        nc.tensor.matmul(PSS[:], ONESC[:], COLP[:], start=True, stop=True)
        nc.vector.reciprocal(SMALL[:, 0:2], PSS[:])   # [1/sum(sens), 1/sum(n)]
        PSB = ps1.tile([P, 2], F32, tag="smallP")
        nc.tensor.matmul(PSB[:], ONESR[:], SMALL[:, 0:2], start=True, stop=True)
        nc.vector.tensor_copy(SCL[:], PSB[:])
        # a = n * (1/sum n)
        nc.vector.tensor_scalar_mul(A[:], NR[:], SCL[:, 1:2])
        # ns = n * sens * (1/sum sens)   (SR <- scaled ns)
        nc.vector.tensor_mul(SR[:], NR[:], SR[:])
        nc.vector.tensor_scalar_mul(SR[:], SR[:], SCL[:, 0:1])
        # D = ns*err - theta   (into PR as scratch), row-min M, E = exp(50*(M-D))
        nc.vector.tensor_mul(r3(PR), bcast_j(SR), r3(EB))
        nc.vector.tensor_sub(PR[:], PR[:], TH[:])
        nc.vector.tensor_reduce(M[:], r3(PR), axis=AXX, op=mybir.AluOpType.min)
        nc.vector.tensor_sub(r3(E), bcast_j(M), r3(PR))
        nc.scalar.activation(E[:], E[:], mybir.ActivationFunctionType.Exp,
                             scale=EPS_INV)
        # b = softmax(phi) (normalized; max-subtracted)
        nc.vector.tensor_reduce(SMALL[:, 2:3], PH[:], axis=AXX,
                                op=mybir.AluOpType.max)
        nc.scalar.mul(SMALL[:, 3:4], SMALL[:, 2:3], -1.0)
        nc.scalar.activation(BT[:], PH[:], mybir.ActivationFunctionType.Exp,
                             bias=SMALL[:, 3:4], scale=1.0)
        nc.vector.tensor_reduce(SMALL[:, 2:3], BT[:], axis=AXX, op=ADD)
        nc.vector.reciprocal(SMALL[:, 2:3], SMALL[:, 2:3])
        nc.vector.tensor_scalar_mul(BT[:], BT[:], SMALL[:, 2:3])

        # ---- 200 sinkhorn iterations ----
        wb_ps = None
        for t in range(iters):
            wb_view = bcast_r(WB0) if t == 0 else \
                wb_ps[:].unsqueeze(1).broadcast_to((P, R, B))
            # S = sum_j E * w
            nc.vector.tensor_mul(r3(PR), r3(E), wb_view)
            nc.vector.tensor_reduce(S[:], r3(PR), axis=AXX, op=ADD)
            # u = a / S
            nc.vector.reciprocal_approx_accurate(RS[:], S[:], SC[:])
            nc.vector.tensor_mul(U[:], A[:], RS[:])
            # T = sum_i E * u  (free-dim partial per partition, then PE reduce)
            nc.vector.tensor_mul(r3(PR2), r3(E), bcast_j(U))
            nc.vector.tensor_reduce(TP[:], c3(PR2), axis=AXX, op=ADD)
            tt = ps.tile([1, B], F32, tag="tt")
            nc.tensor.matmul(tt[:], ONESC[:], TP[:], start=True, stop=True)
            # w = b / T
            nc.vector.reciprocal(RT[:], tt[:])
            nc.vector.tensor_mul(W[:], BT[:], RT[:])
            # broadcast w to all partitions (rank-1 ones outer product)
            wb_ps = ps.tile([P, B], F32, tag="wb")
            nc.tensor.matmul(wb_ps[:], ONESR[:], W[:], start=True, stop=True)

        # ---- P = u*E*w / total ----
        if wb_ps is None:  # iters == 0 (timing-baseline build only)
            nc.gpsimd.memset(PR2[:], 1.0)
            wb_view = bcast_r(WB0)
        else:
            wb_view = wb_ps[:].unsqueeze(1).broadcast_to((P, R, B))
        nc.vector.tensor_mul(r3(PR), r3(PR2), wb_view)
        nc.vector.tensor_reduce(COLP[:, 0:1], PR[:], axis=AXX, op=ADD)
        PT = ps1.tile([1, 2], F32, tag="small1")
        nc.tensor.matmul(PT[:, 0:1], ONESC[:], COLP[:, 0:1], start=True, stop=True)
        nc.vector.reciprocal(SMALL[:, 0:1], PT[:, 0:1])
        PTB = ps1.tile([P, 2], F32, tag="smallP")
        nc.tensor.matmul(PTB[:, 0:1], ONESR[:], SMALL[:, 0:1], start=True, stop=True)
        nc.vector.tensor_copy(SCL[:, 0:1], PTB[:, 0:1])
        nc.vector.tensor_scalar_mul(PR[:], PR[:], SCL[:, 0:1])
        nc.sync.dma_start(out_d.rearrange("(p r) j -> p (r j)", p=P), PR[:])

    nc.compile()
    return nc


_cache = {}


def _get_nc(iters=ITERS):
    if iters not in _cache:
        _cache[iters] = build(iters)
    return _cache[iters]


def kernel(**inputs):
    nc = _get_nc()
    in_map = {
        "theta": np.ascontiguousarray(inputs["theta"], dtype=np.float32),
        "phi": np.ascontiguousarray(inputs["phi"], dtype=np.float32),
        "sens_raw": np.ascontiguousarray(inputs["sens_raw"], dtype=np.float32),
        "n_raw": np.ascontiguousarray(inputs["n_raw"], dtype=np.float32),
    }
    res = run_bass_kernel_spmd(nc, [dict(in_map) for _ in range(8)],
                               list(range(8)))
    return np.asarray(res.results[0]["out"], dtype=np.float32)


# revision 13
# speedup vs baseline: 9.6297x; 9.6297x over previous
"""Trainium2 Bass kernel for the DifferentiableAllocator (Sinkhorn) problem.

Math: the reference runs 200 log-domain Sinkhorn iterations on K = -(C-theta)/eps
with row marginal a and column marginal b.  With E = exp(K - rowmax(K)) and the
identities u = exp(f + rowmax(K)) = a / (E w),  w = exp(g) = b / (E^T u), the
whole loop collapses to scaled matrix-balancing with NO transcendentals:

    w0 = 1;  repeat 200x:  S = E w;  u = a/S;  T = E^T u;  w = b/T
    P = diag(u) E diag(w) / sum(...)

which is exactly what the reference computes (gauge-invariant; validated to
rel err ~5.6e-5 against the fp32 jax reference, 3.5e-6 against fp64).

Folding the constant marginals into the kernel (F = E*b, G = E*a) and writing
the recurrence in terms of rT = 1/T reduces each iteration to 6 DVE ops and
one PE matmul:

    S   = sum_j F[:, j] * rT[j]          (TT mult + grouped free-dim reduce)
    rS  = approx_recip(S)                (1 custom-DVE op; noise is damped
                                          by the Sinkhorn contraction)
    TP  = sum_r G[:, r, :] * rS[:, r]    (TT mult + grouped free-dim reduce)
    TB  = ONES[128x128] @ TP             (one matmul = partition-sum AND
                                          broadcast of T to all partitions)
    rT  = approx_recip(TB)

Layout: rows L=8192 fold onto 128 partitions x 64 rows, 8 columns contiguous
in the free dim.  Runs replicated on all 8 cores (a single serial 200-step
chain over 256 KB of state - sharding would add an allreduce per iteration
for no win); core 0's output is returned.
"""

import numpy as np
from contextlib import ExitStack

import concourse.bass as bass
import concourse.bacc as bacc
import concourse.tile as tile
from concourse import mybir
from concourse.bass_utils import run_bass_kernel_spmd

L, B = 8192, 8
P = 128
R = L // P  # 64 rows per partition
ITERS = 200
EPS_INV = 50.0  # 1/0.02
BITS = (2.0, 3.0, 4.0, 5.0, 6.0, 8.0, 10.0, 12.0)
F32 = mybir.dt.float32
ADD = mybir.AluOpType.add
AXX = mybir.AxisListType.X


def build(iters=ITERS):
    nc = bacc.Bacc("TRN2", target_bir_lowering=False, debug=False)
    theta_d = nc.dram_tensor("theta", [L, B], F32, kind="ExternalInput").ap()
    phi_d = nc.dram_tensor("phi", [B], F32, kind="ExternalInput").ap()
    sens_d = nc.dram_tensor("sens_raw", [L], F32, kind="ExternalInput").ap()
    nraw_d = nc.dram_tensor("n_raw", [L], F32, kind="ExternalInput").ap()
    out_d = nc.dram_tensor("out", [L, B], F32, kind="ExternalOutput").ap()

    with tile.TileContext(nc) as tc, ExitStack() as ctx:
        sb = ctx.enter_context(tc.tile_pool(name="sb", bufs=1))
        ps = ctx.enter_context(tc.tile_pool(name="ps", bufs=2, space="PSUM"))
        ps1 = ctx.enter_context(tc.tile_pool(name="ps1", bufs=1, space="PSUM"))

        # ---- persistent tiles ----
        TH = sb.tile([P, R * B], F32, tag="TH")    # theta
        E = sb.tile([P, R * B], F32, tag="E")      # exp(K - rowmax K)
        F = sb.tile([P, R * B], F32, tag="F")      # E * b
        G = sb.tile([P, R * B], F32, tag="G")      # E * a
        PR = sb.tile([P, R * B], F32, tag="PR")    # row-pass product / scratch
        PR2 = sb.tile([P, R * B], F32, tag="PR2")  # col-pass product
        EB = sb.tile([P, R * B], F32, tag="EB")    # err broadcast
        NR = sb.tile([P, R], F32, tag="NR")        # n_raw -> n
        SR = sb.tile([P, R], F32, tag="SR")        # sens_raw -> n*sens scaled
        A = sb.tile([P, R], F32, tag="A")          # a = n / sum(n)
        S = sb.tile([P, R], F32, tag="S")
        RS = sb.tile([P, R], F32, tag="RS")        # 1/S
        M = sb.tile([P, R], F32, tag="M")          # row min of D
        TP = sb.tile([P, B], F32, tag="TP")        # per-partition col partials
        RTB = sb.tile([P, B], F32, tag="RTB")      # 1/T broadcast on partitions
        BB = sb.tile([P, B], F32, tag="BB")        # b broadcast on partitions
        BT = sb.tile([1, B], F32, tag="BT")        # b (softmax(phi))
        RB = sb.tile([1, B], F32, tag="RB")        # 1/b
        PH = sb.tile([1, B], F32, tag="PH")
        SMALL = sb.tile([1, 4], F32, tag="SMALL")  # misc 1x1 values
        COLP = sb.tile([P, 2], F32, tag="COLP")    # per-partition partials
        SCL = sb.tile([P, 2], F32, tag="SCL")      # per-partition scalars
        ONESC = sb.tile([P, 1], F32, tag="ONESC")  # ones column (matmul lhsT)
        ONESR = sb.tile([1, P], F32, tag="ONESR")  # ones row (matmul lhsT)
        ONES2 = sb.tile([P, P], F32, tag="ONES2")  # ones 128x128 (matmul lhsT)

        def r3(t):  # [P, R*B] -> [P, R, B]
            return t[:].rearrange("p (r j) -> p r j", j=B)

        def c3(t):  # [P, R*B] -> [P, B, R]  (view for per-column reduce)
            return t[:].rearrange("p (r j) -> p j r", j=B)

        def bcast_j(t2):  # [P, R] -> [P, R, B] (step-0 over j)
            return t2[:].unsqueeze(2).broadcast_to((P, R, B))

        def bcast_r(ap2):  # [P, B] AP -> [P, R, B] (step-0 over r)
            return ap2.unsqueeze(1).broadcast_to((P, R, B))

        # ---- loads ----
        nc.sync.dma_start(TH[:], theta_d.rearrange("(p r) j -> p (r j)", p=P))
        nc.sync.dma_start(NR[:], nraw_d.rearrange("(p r) -> p r", p=P))
        nc.sync.dma_start(SR[:], sens_d.rearrange("(p r) -> p r", p=P))
        nc.sync.dma_start(PH[:], phi_d.unsqueeze(0))

        # ---- constants ----
        nc.gpsimd.memset(ONESC[:], 1.0)
        nc.gpsimd.memset(ONESR[:], 1.0)
        nc.gpsimd.memset(ONES2[:], 1.0)
        for j, bits in enumerate(BITS):
            nc.gpsimd.memset(r3(EB)[:, :, j], float(2.0 ** (-2.0 * bits)))

        # ---- setup: n, sens, a, b, E, F, G ----
        # n = n_raw * 1e5 + 1e3
        nc.vector.tensor_scalar(NR[:], NR[:], 1e5, 1e3,
                                op0=mybir.AluOpType.mult, op1=ADD)
        # 1/sum(sens), 1/sum(n): free-reduce, ones-matmul, recip, broadcast
        nc.vector.tensor_reduce(COLP[:, 0:1], SR[:], axis=AXX, op=ADD)
        nc.vector.tensor_reduce(COLP[:, 1:2], NR[:], axis=AXX, op=ADD)
        PSS = ps1.tile([1, 2], F32, tag="small1")
        nc.tensor.matmul(PSS[:], ONESC[:], COLP[:], start=True, stop=True)
        nc.vector.reciprocal(SMALL[:, 0:2], PSS[:])
        PSB = ps1.tile([P, 2], F32, tag="smallP")
        nc.tensor.matmul(PSB[:], ONESR[:], SMALL[:, 0:2], start=True, stop=True)
        nc.vector.tensor_copy(SCL[:], PSB[:])
        # a = n * (1/sum n);  ns = n * sens * (1/sum sens)
        nc.vector.tensor_scalar_mul(A[:], NR[:], SCL[:, 1:2])
        nc.vector.tensor_mul(SR[:], NR[:], SR[:])
        nc.vector.tensor_scalar_mul(SR[:], SR[:], SCL[:, 0:1])
        # D = ns*err - theta (in PR), M = rowmin D, E = exp(50*(M - D))
        nc.vector.tensor_mul(r3(PR), bcast_j(SR), r3(EB))
        nc.vector.tensor_sub(PR[:], PR[:], TH[:])
        nc.vector.tensor_reduce(M[:], r3(PR), axis=AXX, op=mybir.AluOpType.min)
        nc.vector.tensor_sub(r3(E), bcast_j(M), r3(PR))
        nc.scalar.activation(E[:], E[:], mybir.ActivationFunctionType.Exp,
                             scale=EPS_INV)
        # b = softmax(phi)
        nc.vector.tensor_reduce(SMALL[:, 2:3], PH[:], axis=AXX,
                                op=mybir.AluOpType.max)
        nc.scalar.mul(SMALL[:, 3:4], SMALL[:, 2:3], -1.0)
        nc.scalar.activation(BT[:], PH[:], mybir.ActivationFunctionType.Exp,
                             bias=SMALL[:, 3:4], scale=1.0)
        nc.vector.tensor_reduce(SMALL[:, 2:3], BT[:], axis=AXX, op=ADD)
        nc.vector.reciprocal(SMALL[:, 2:3], SMALL[:, 2:3])
        nc.vector.tensor_scalar_mul(BT[:], BT[:], SMALL[:, 2:3])
        nc.vector.reciprocal(RB[:], BT[:])
        # broadcast b and 1/b to all partitions; F = E*b, G = E*a, rT0 = 1/b
        PBB = ps1.tile([P, B], F32, tag="pbb")
        nc.tensor.matmul(PBB[:], ONESR[:], BT[:], start=True, stop=True)
        nc.vector.tensor_copy(BB[:], PBB[:])
        PRB = ps1.tile([P, B], F32, tag="prb")
        nc.tensor.matmul(PRB[:], ONESR[:], RB[:], start=True, stop=True)
        nc.vector.tensor_copy(RTB[:], PRB[:])
        nc.vector.tensor_mul(r3(F), r3(E), bcast_r(BB[:]))
        nc.vector.tensor_mul(r3(G), r3(E), bcast_j(A))

        # ---- sinkhorn iterations ----
        for t in range(iters):
            # S = sum_j F * rT
            nc.vector.tensor_mul(r3(PR), r3(F), bcast_r(RTB[:]))
            nc.vector.tensor_reduce(S[:], r3(PR), axis=AXX, op=ADD)
            nc.vector.reciprocal_approx_fast(RS[:], S[:])
            # T = sum_i G * rS ; TB = ones @ TP sums partitions + broadcasts
            nc.vector.tensor_mul(r3(PR2), r3(G), bcast_j(RS))
            nc.vector.tensor_reduce(TP[:], c3(PR2), axis=AXX, op=ADD)
            tb = ps.tile([P, B], F32, tag="tb")
            nc.tensor.matmul(tb[:], ONES2[:], TP[:], start=True, stop=True)
            nc.vector.reciprocal_approx_fast(RTB[:], tb[:])

        # ---- P = (G*rS) * (b*rT) / total ----
        if iters == 0:  # timing-baseline build only
            nc.gpsimd.memset(PR2[:], 1.0)
        nc.vector.tensor_mul(r3(PR), r3(PR2), bcast_r(RTB[:]))
        nc.vector.tensor_mul(r3(PR), r3(PR), bcast_r(BB[:]))
        nc.vector.tensor_reduce(COLP[:, 0:1], PR[:], axis=AXX, op=ADD)
        PT = ps1.tile([1, 2], F32, tag="small1")
        nc.tensor.matmul(PT[:, 0:1], ONESC[:], COLP[:, 0:1], start=True, stop=True)
        nc.vector.reciprocal(SMALL[:, 0:1], PT[:, 0:1])
        PTB = ps1.tile([P, 2], F32, tag="smallP")
        nc.tensor.matmul(PTB[:, 0:1], ONESR[:], SMALL[:, 0:1], start=True, stop=True)
        nc.vector.tensor_copy(SCL[:, 0:1], PTB[:, 0:1])
        nc.vector.tensor_scalar_mul(PR[:], PR[:], SCL[:, 0:1])
        nc.sync.dma_start(out_d.rearrange("(p r) j -> p (r j)", p=P), PR[:])

    nc.compile()
    return nc


_cache = {}


def _get_nc(iters=ITERS):
    if iters not in _cache:
        _cache[iters] = build(iters)
    return _cache[iters]


def kernel(**inputs):
    nc = _get_nc()
    in_map = {
        "theta": np.ascontiguousarray(inputs["theta"], dtype=np.float32),
        "phi": np.ascontiguousarray(inputs["phi"], dtype=np.float32),
        "sens_raw": np.ascontiguousarray(inputs["sens_raw"], dtype=np.float32),
        "n_raw": np.ascontiguousarray(inputs["n_raw"], dtype=np.float32),
    }
    res = run_bass_kernel_spmd(nc, [dict(in_map) for _ in range(8)],
                               list(range(8)))
    return np.asarray(res.results[0]["out"], dtype=np.float32)
